# revision 9
# baseline (speedup 1.0000x reference)
"""Multi-head self-attention (B=4, S=2048, D=1024, H=16) on 8 TRN2 NeuronCores.

Sharding: data-parallel over batch x tensor-parallel over heads (Megatron
column-split of w_qkv, row-split of w_out). Core c computes batch c//2 with
heads (c%2)*8..(c%2)*8+8 and produces a partial [S, D] output; the host sums
the two partials per batch and adds the bias.

Per-core kernel (single Tile program, bf16 matmuls ~5e-3 rel err):
  - upfront: x rows DMA'd, cast to bf16, PE-transposed into a resident SBUF
    xT [d, S]; v = x @ wv (seq-major, with a ones column per head for the
    softmax denominators); pair-0 qT/kT projected for the full sequence.
  - the 16 attention passes (4 head-pairs x 2 q-halves x 2 parities, q-half
    outer) run as ONE seamless k-tile stream: per k-tile the PE does
    QK (2 matmuls) + the PV pair two k-tiles behind, and the ACT engine
    exp-s the previous score tile; consecutive passes overlap (the first two
    k-tiles of pass p+1 carry the trailing PVs of pass p) so neither engine
    drains at a boundary.
  - remaining work (later pairs' projections, out-projection rows) lives in
    a prerequisite-tagged FIFO sprinkled one item every few k-tiles to fill
    the PE's per-k-tile slack (ACT exp 1114ns vs 852ns of attention matmuls).
  - normalize: 1/denom (DVE) -> partition_broadcast (GPSIMD) -> multiply into
    the feat-major outT tile; PSUM->SBUF drains ride the idle GPSIMD engine.
  - y = sum_pairs outT^T @ wout at K=128, sprinkled through the second
    q-half's passes.
"""

import numpy as np

from concourse import bass_utils



from collections import deque
from contextlib import ExitStack

import concourse.bacc as bacc
import concourse.bass as bass
import concourse.mybir as mybir
import concourse.tile as tile
from concourse import masks

P = 128
HD = 64
HV = HD + 1
QCH = 512
F32 = mybir.dt.float32
F32R = mybir.dt.float32r
BF16 = mybir.dt.bfloat16
EXP = mybir.ActivationFunctionType.Exp


def build_attention(
    S: int,
    D: int,
    HN: int,
    DO: int,
    scale: float,
    dt_x=BF16,
    dt_qk=BF16,
    dt_e=BF16,
    dt_o=BF16,
) -> bacc.Bacc:
    F = HN * HD
    n_st = S // P
    n_dt = D // P
    n_ft = F // P
    n_ch = S // QCH
    n_kt = S // P
    n_no = DO // QCH
    QH = min(1024, S)
    n_qh = S // QH
    n_j = QH // QCH
    n_sti = QCH // P
    assert S % QCH == 0 and D % P == 0 and F % P == 0 and DO % QCH == 0
    assert mybir.dt.size(dt_x) == 2, "this build is bf16-only"

    nc = bacc.Bacc("TRN2", target_bir_lowering=False, debug=False)

    x = nc.dram_tensor("x", [S, D], F32, kind="ExternalInput")
    wq = nc.dram_tensor("wq", [D, F], F32, kind="ExternalInput")
    wk = nc.dram_tensor("wk", [D, F], F32, kind="ExternalInput")
    wv = nc.dram_tensor("wv", [D, F], F32, kind="ExternalInput")
    wout = nc.dram_tensor("wout", [F, DO], F32, kind="ExternalInput")
    y = nc.dram_tensor("y", [S, DO], F32, kind="ExternalOutput")

    with tile.TileContext(nc) as tc, ExitStack() as top:  # noqa: PLR1702
        const_pool = top.enter_context(tc.tile_pool(name="const", bufs=1))
        ident = const_pool.tile([P, P], F32, tag="ident")
        masks.make_identity(nc, ident[:])
        ident_b = const_pool.tile([P, P], BF16, tag="identb")
        nc.vector.tensor_copy(ident_b[:], ident[:])
        ones_f32 = const_pool.tile([P, HD], F32, tag="ones_f32")
        nc.gpsimd.memset(ones_f32[:], 1.0)

        v_pool = top.enter_context(tc.tile_pool(name="vsb", bufs=1))
        v_sb = [
            v_pool.tile([P, HN * HV], dt_e, tag=f"v{st}", name=f"v_sb{st}")
            for st in range(n_st)
        ]
        for st in range(n_st):
            nc.vector.tensor_copy(
                v_sb[st][:].rearrange("p (h v) -> p h v", v=HV)[:, :, HD:].rearrange(
                    "p h one -> p (h one)"
                ),
                ones_f32[:, :HN],
            )

        outT_pool = top.enter_context(tc.tile_pool(name="outT", bufs=1))
        outP = [
            outT_pool.tile([P, S], dt_o, tag=f"o{ft}", name=f"outP{ft}")
            for ft in range(n_ft)
        ]

        wqk_pool = top.enter_context(tc.tile_pool(name="wqk", bufs=1))
        # one slab per (pair, q|k): [128, n_dt*128] with free = (db, col) —
        # a single DMA instead of n_dt small ones (SWDGE queue serializes)
        wq_s = [
            wqk_pool.tile([P, n_dt * P], dt_x, tag=f"wqs{ft}", name=f"wqs{ft}")
            for ft in range(n_ft)
        ]
        wk_s = [
            wqk_pool.tile([P, n_dt * P], dt_x, tag=f"wks{ft}", name=f"wks{ft}")
            for ft in range(n_ft)
        ]
        wq_t = [[wq_s[ft][:, db * P : (db + 1) * P] for ft in range(n_ft)] for db in range(n_dt)]
        wk_t = [[wk_s[ft][:, db * P : (db + 1) * P] for ft in range(n_ft)] for db in range(n_dt)]

        def load_w_pair(ft):
            nc.gpsimd.dma_start(
                wq_s[ft][:].rearrange("p (db c) -> p db c", c=P),
                wq[:, ft * P : (ft + 1) * P].rearrange("(db p) c -> p db c", p=P),
            )
            nc.gpsimd.dma_start(
                wk_s[ft][:].rearrange("p (db c) -> p db c", c=P),
                wk[:, ft * P : (ft + 1) * P].rearrange("(db p) c -> p db c", p=P),
            )

        # all four pairs stay resident (q-half-outer pass order reuses them)
        pair_pool = top.enter_context(tc.tile_pool(name="pair", bufs=n_ft))
        pair_tiles = {}

        def get_pair(ft):
            if ft not in pair_tiles:
                pair_tiles[ft] = (
                    pair_pool.tile([P, S], dt_qk, tag="qp", name=f"qTp{ft}"),
                    pair_pool.tile([P, S], dt_qk, tag="kp", name=f"kTp{ft}"),
                )
            return pair_tiles[ft]

        wv_pool = top.enter_context(tc.tile_pool(name="wvp", bufs=1))
        wv_t = [
            wv_pool.tile([P, F], dt_x, tag=f"wv{db}", name=f"wv{db}")
            for db in range(n_dt)
        ]
        # x^T kept resident in SBUF for the whole run (no DRAM roundtrip)
        xT_pool = top.enter_context(tc.tile_pool(name="xTsb", bufs=1))
        xT_sb = [
            xT_pool.tile([P, S], dt_x, tag=f"xT{db}", name=f"xTsb{db}")
            for db in range(n_dt)
        ]
        # upfront weight DMAs: only what the upfront phase + first passes
        # need (pair0, wv, pair1); pairs 2-3 + wout are issued later so they
        # don't compete with the x loads for HBM bandwidth
        load_w_pair(0)
        for db in range(n_dt):
            nc.gpsimd.dma_start(wv_t[db][:], wv[db * P : (db + 1) * P, :])
        load_w_pair(1)

        ps_sc = top.enter_context(
            tc.tile_pool(name="ps_sc", bufs=3, space=bass.MemorySpace.PSUM)
        )
        ps_pv = top.enter_context(
            tc.tile_pool(name="ps_pv", bufs=1, space=bass.MemorySpace.PSUM)
        )
        e_pool = top.enter_context(tc.tile_pool(name="epool", bufs=4))
        stg_pool = top.enter_context(tc.tile_pool(name="stgpool", bufs=3))
        rc_pool = top.enter_context(tc.tile_pool(name="rcpool", bufs=2))
        bcs_pool = top.enter_context(tc.tile_pool(name="bcspool", bufs=2))
        # x row staging is only needed during the upfront transpose phase;
        # created last so it can be released (stack order) before wo/ys
        xup_stack = ExitStack()
        xst_pool = xup_stack.enter_context(tc.tile_pool(name="xst", bufs=2 * n_sti))

        # ---------------- building blocks ----------------
        def upfront_chunk(ch, qTp0, kTp0):
            xrows = []
            for sti in range(n_sti):
                st = ch * n_sti + sti
                xrow = xst_pool.tile([P, D], F32, tag="xrow", name=f"xrow{st}")
                nc.sync.dma_start(xrow[:], x[st * P : (st + 1) * P, :])
                xb = xst_pool.tile(
                    [P, D], dt_x, tag="xbf", bufs=2 * n_sti, name=f"xb{st}"
                )
                nc.vector.tensor_copy(xb[:], xrow[:])
                xrows.append(xb)
            xT = [xT_sb[db][:, ch * QCH : (ch + 1) * QCH] for db in range(n_dt)]
            for db in range(n_dt):
                tp = ps_sc.tile([P, QCH], dt_x, tag="sc", name=f"tr{ch}_{db}")
                for sti in range(n_sti):
                    nc.tensor.transpose(
                        tp[:, sti * P : (sti + 1) * P],
                        xrows[sti][:, db * P : (db + 1) * P],
                        ident_b[:],
                    )
                # PSUM->SBUF drain on the ACT engine (idle during upfront;
                # GPSIMD cannot access PSUM)
                nc.scalar.copy(xT[db], tp[:])
            for w_t, dstp in ((wq_t, qTp0), (wk_t, kTp0)):
                pp = ps_sc.tile([P, QCH], F32, tag="sc", name=f"pj0_{ch}")
                for db in range(n_dt):
                    nc.tensor.matmul(
                        pp[:],
                        w_t[db][0],
                        xT[db],
                        start=(db == 0),
                        stop=(db == n_dt - 1),
                    )
                nc.vector.tensor_copy(dstp[:, ch * QCH : (ch + 1) * QCH], pp[:])
            for sti in range(n_sti):
                st = ch * n_sti + sti
                pv_ps = ps_sc.tile([P, F], F32, tag="sc", name=f"pvp{st}")
                for db in range(n_dt):
                    nc.tensor.matmul(
                        pv_ps[:],
                        xT_sb[db][:, st * P : (st + 1) * P],
                        wv_t[db][:],
                        start=(db == 0),
                        stop=(db == n_dt - 1),
                    )
                nc.vector.tensor_copy(
                    v_sb[st][:].rearrange("p (h v) -> p h v", v=HV)[:, :, :HD],
                    pv_ps[:].rearrange("p (h d) -> p h d", d=HD),
                )

        def proj_items(ftn):
            """Matmul closures projecting pair ftn's qT/kT from resident xT."""
            qTp, kTp = get_pair(ftn)

            def mm_item(ch, w_t, dstp, which):
                def run():
                    pp = ps_sc.tile([P, QCH], F32, tag="sc", name=f"pj{which}{ftn}_{ch}")
                    for db in range(n_dt):
                        nc.tensor.matmul(
                            pp[:],
                            w_t[db][ftn],
                            xT_sb[db][:, ch * QCH : (ch + 1) * QCH],
                            start=(db == 0),
                            stop=(db == n_dt - 1),
                        )
                    nc.vector.tensor_copy(dstp[:, ch * QCH : (ch + 1) * QCH], pp[:])

                return run

            items = []
            for ch in range(n_ch):
                items.append(mm_item(ch, wk_t, kTp, "k"))
                items.append(mm_item(ch, wq_t, qTp, "q"))
            return items

        class AttnQH:
            """Emitter for one (pair, q-half, head-parity) attention pass,
            driven one k-tile at a time by the global stream. PV trails QK by
            LAG k-tiles so the ACT exp chain never stalls the PE."""

            LAG = 2

            def __init__(self, ft, qh, parity):
                self.ft, self.qh, self.parity = ft, qh, parity
                self.qTp, self.kTp = get_pair(ft)
                self.h = 2 * ft + parity
                self.q_base = qh * QH
                self.pv = ps_pv.tile(
                    [HV, QH], F32, tag="pv", name=f"pv{ft}_{qh}_{parity}"
                )
                self.prevs = deque()

            def emit_qk(self, kt):
                sub = self.parity * HD
                sc = ps_sc.tile(
                    [P, QH], F32, tag="sc",
                    name=f"sc{self.ft}{self.parity}{self.qh}{kt}",
                )
                for j in range(n_j):
                    q0 = self.q_base + j * QCH
                    nc.tensor.matmul(
                        sc[:, j * QCH : (j + 1) * QCH],
                        self.kTp[sub : sub + HD, kt * P : (kt + 1) * P],
                        self.qTp[sub : sub + HD, q0 : q0 + QCH],
                        start=True,
                        stop=True,
                    )
                et = e_pool.tile(
                    [P, QH], dt_e, tag="et",
                    name=f"e{self.ft}{self.parity}{self.qh}{kt}",
                )
                nc.scalar.activation(et[:], sc[:], EXP, scale=scale)
                self.prevs.append((kt, et))

            def emit_pv_one(self):
                kt, et = self.prevs.popleft()
                vt = v_sb[kt][:].rearrange("p (hh v) -> p hh v", v=HV)[:, self.h, :]
                for j in range(n_j):
                    nc.tensor.matmul(
                        self.pv[:, j * QCH : (j + 1) * QCH],
                        vt,
                        et[:, j * QCH : (j + 1) * QCH],
                        start=(kt == 0),
                        stop=(kt == n_kt - 1),
                    )
                return not self.prevs

            def finish_stage1(self):
                """Copy pv to SBUF staging (frees the PSUM accumulator)."""
                ft, qh, parity = self.ft, self.qh, self.parity
                self.stg = stg_pool.tile(
                    [HV, QH], F32, tag="stg", name=f"st{ft}{parity}{qh}"
                )
                nc.vector.tensor_copy(self.stg[:], self.pv[:])

            def normalize_items(self):
                """Per-chunk normalize closures (reciprocal + broadcast +
                multiply); DVE/GPSIMD work, no PE instructions."""
                ft, qh, parity, q_base = self.ft, self.qh, self.parity, self.q_base
                stg = self.stg

                def norm_item(qc):
                    def run():
                        rc = rc_pool.tile(
                            [1, QCH], F32, tag="rc", name=f"rc{ft}{parity}{qh}{qc}"
                        )
                        nc.vector.reciprocal(
                            rc[:], stg[HD : HD + 1, qc * QCH : (qc + 1) * QCH]
                        )
                        bcs = bcs_pool.tile(
                            [HD, QCH], F32, tag="bcs", name=f"bc{ft}{parity}{qh}{qc}"
                        )
                        nc.gpsimd.partition_broadcast(bcs[:], rc[:])
                        with nc.allow_low_precision(reason="attn out cast"):
                            nc.vector.tensor_mul(
                                outP[ft][
                                    parity * HD : (parity + 1) * HD,
                                    q_base + qc * QCH : q_base + (qc + 1) * QCH,
                                ],
                                stg[:HD, qc * QCH : (qc + 1) * QCH],
                                bcs[:],
                            )

                    return run

                return [norm_item(qc) for qc in range(n_j)]

        def y_items(qt_range, wo_t, ys_pool):
            def y_item(qt):
                def run():
                    for no in range(n_no):
                        yp = ps_sc.tile([P, QCH], F32, tag="sc", name=f"yp{qt}_{no}")
                        for ft in range(n_ft):
                            nc.tensor.matmul(
                                yp[:],
                                outP[ft][:, qt * P : (qt + 1) * P],
                                wo_t[ft][:, no * QCH : (no + 1) * QCH],
                                start=(ft == 0),
                                stop=(ft == n_ft - 1),
                            )
                        ys = ys_pool.tile([P, QCH], F32, tag="ys", name=f"ys{qt}_{no}")
                        nc.vector.tensor_copy(ys[:], yp[:])
                        nc.sync.dma_start(
                            y[qt * P : (qt + 1) * P, no * QCH : (no + 1) * QCH], ys[:]
                        )

                return run

            return [y_item(qt) for qt in qt_range]

        # ---------------- emission ----------------
        qTp0, kTp0 = get_pair(0)
        for ch in range(n_ch):
            upfront_chunk(ch, qTp0, kTp0)
        xup_stack.close()

        # late weight DMAs: pairs 2..n_ft-1 and wout
        for ftn in range(2, n_ft):
            load_w_pair(ftn)
        wo_pool = top.enter_context(tc.tile_pool(name="wo", bufs=1))
        ys_pool = top.enter_context(tc.tile_pool(name="ys", bufs=3))
        wo_t = [
            wo_pool.tile([P, DO], dt_o, tag=f"wo{ft2}", name=f"wo{ft2}")
            for ft2 in range(n_ft)
        ]
        for ft2 in range(n_ft):
            nc.gpsimd.dma_start(wo_t[ft2][:], wout[ft2 * P : (ft2 + 1) * P, :])

        # global sprinkle FIFO of (must_emit_before_pass_idx, closure)
        work = deque()
        for ftn in range(1, n_ft):
            for it in proj_items(ftn):
                work.append((2 * ftn, it))

        rows_per_qh = n_st // n_qh
        pass_specs = [
            (qh, ft, parity)
            for qh in range(n_qh)
            for ft in range(n_ft)
            for parity in (0, 1)
        ]
        n_passes = len(pass_specs)
        NEVER = n_passes + 1

        prev = None
        pending_norms = []
        enqueue_next = []  # items appended to `work` at the next pass boundary
        for pidx, (qh, ft, parity) in enumerate(pass_specs):
            for it in enqueue_next:
                work.append((NEVER, it))
            enqueue_next = []
            # force-emit overdue prerequisites (pair projections)
            while work and work[0][0] <= pidx:
                work.popleft()[1]()
            a = AttnQH(ft, qh, parity)
            stride = 4 if qh == 0 else 14
            for kt in range(n_kt):
                if pending_norms and kt >= 2:
                    pending_norms.pop(0)()
                if work and kt % stride == stride - 1:
                    work.popleft()[1]()
                a.emit_qk(kt)
                if prev is not None:
                    if prev.emit_pv_one():
                        prev.finish_stage1()
                        pending_norms += prev.normalize_items()
                        if (prev.qh, prev.ft, prev.parity) == (0, n_ft - 1, 1):
                            # first q-half fully normalized soon: release its
                            # out-projection rows into the stream
                            enqueue_next += y_items(
                                range(rows_per_qh), wo_t, ys_pool
                            )
                        prev = None
                elif len(a.prevs) > AttnQH.LAG:
                    a.emit_pv_one()
            prev = a

        # tail: flush the last pass, run leftovers, last normalizes, last rows
        while prev.prevs:
            if work:
                work.popleft()[1]()
            prev.emit_pv_one()
        prev.finish_stage1()
        tail_norms = pending_norms + prev.normalize_items()
        leftovers = [it for _, it in work]
        for i, it in enumerate(leftovers[:2]):
            if tail_norms:
                tail_norms.pop(0)()
            it()
        for it in tail_norms:
            it()
        for it in leftovers[2:]:
            it()
        for it in y_items(range(rows_per_qh, n_st), wo_t, ys_pool):
            it()

    nc.compile()
    return nc


# problem sizes (hardcoded per contract)
B, S, D, H = 4, 2048, 1024, 16
DO = D
HN = H // 2  # heads per core
SCALE = (D // H) ** -0.5
N_CORES = 8

_NC_CACHE = None


def _get_nc():
    global _NC_CACHE
    if _NC_CACHE is None:
        _NC_CACHE = build_attention(S, D, HN, DO, SCALE)
    return _NC_CACHE


def make_in_maps(x, w_qkv, w_out):
    """Shard full inputs into the 8 per-core input maps."""
    in_maps = []
    for c in range(N_CORES):
        b = c // 2
        cs = (c % 2) * HN * HD
        ce = cs + HN * HD
        in_maps.append(
            {
                "x": np.ascontiguousarray(x[b]),
                "wq": np.ascontiguousarray(w_qkv[:, cs:ce]),
                "wk": np.ascontiguousarray(w_qkv[:, D + cs : D + ce]),
                "wv": np.ascontiguousarray(w_qkv[:, 2 * D + cs : 2 * D + ce]),
                "wout": np.ascontiguousarray(w_out[cs:ce, :]),
            }
        )
    return in_maps


def combine_outputs(results, b_out):
    """Sum the two per-batch partials and add the bias."""
    y = np.empty((B, S, DO), dtype=np.float32)
    for b in range(B):
        y[b] = results[2 * b]["y"] + results[2 * b + 1]["y"] + b_out[None, :]
    return y


def kernel(x, w_qkv, w_out, b_out):
    x = np.asarray(x, dtype=np.float32)
    w_qkv = np.asarray(w_qkv, dtype=np.float32)
    w_out = np.asarray(w_out, dtype=np.float32)
    b_out = np.asarray(b_out, dtype=np.float32)
    nc = _get_nc()
    in_maps = make_in_maps(x, w_qkv, w_out)
    res = bass_utils.run_bass_kernel_spmd(nc, in_maps, core_ids=list(range(N_CORES)))
    return combine_outputs(res.results, b_out)


# revision 14
# speedup vs baseline: 1.0301x; 1.0301x over previous
"""Multi-head self-attention (B=4, S=2048, D=1024, H=16) on 8 TRN2 NeuronCores.

Sharding: data-parallel over batch x tensor-parallel over heads (Megatron
column-split of w_qkv, row-split of w_out). Core c computes batch c//2 with
heads (c%2)*8..(c%2)*8+8 and produces a partial [S, D] output; the host sums
the two partials per batch and adds the bias.

Per-core kernel (single Tile program, bf16 matmuls ~5e-3 rel err):
  - upfront: x rows DMA'd, cast to bf16, PE-transposed into a resident SBUF
    xT [d, S]; v = x @ wv (seq-major, with a ones column per head for the
    softmax denominators); pair-0 qT/kT projected for the full sequence.
  - the 16 attention passes (4 head-pairs x 2 q-halves x 2 parities, q-half
    outer) run as ONE seamless k-tile stream: per k-tile the PE does
    QK (2 matmuls) + the PV pair two k-tiles behind, and the ACT engine
    exp-s the previous score tile; consecutive passes overlap (the first two
    k-tiles of pass p+1 carry the trailing PVs of pass p) so neither engine
    drains at a boundary.
  - remaining work (later pairs' projections, out-projection rows) lives in
    a prerequisite-tagged FIFO sprinkled one item every few k-tiles to fill
    the PE's per-k-tile slack (ACT exp 1114ns vs 852ns of attention matmuls).
  - normalize: 1/denom (DVE) -> partition_broadcast (GPSIMD) -> multiply into
    the feat-major outT tile; PSUM->SBUF drains ride the idle GPSIMD engine.
  - y = sum_pairs outT^T @ wout at K=128, sprinkled through the second
    q-half's passes.
"""

import numpy as np

from concourse import bass_utils



from collections import deque
from contextlib import ExitStack

import concourse.bacc as bacc
import concourse.bass as bass
import concourse.mybir as mybir
import concourse.tile as tile
from concourse import masks

P = 128
HD = 64
HV = HD + 1
QCH = 512
F32 = mybir.dt.float32
F32R = mybir.dt.float32r
BF16 = mybir.dt.bfloat16
EXP = mybir.ActivationFunctionType.Exp


def build_attention(
    S: int,
    D: int,
    HN: int,
    DO: int,
    scale: float,
    dt_x=BF16,
    dt_qk=BF16,
    dt_e=BF16,
    dt_o=BF16,
) -> bacc.Bacc:
    F = HN * HD
    n_st = S // P
    n_dt = D // P
    n_ft = F // P
    n_ch = S // QCH
    n_kt = S // P
    n_no = DO // QCH
    QH = min(1024, S)
    n_qh = S // QH
    n_j = QH // QCH
    n_sti = QCH // P
    assert S % QCH == 0 and D % P == 0 and F % P == 0 and DO % QCH == 0
    assert mybir.dt.size(dt_x) == 2, "this build is bf16-only"

    nc = bacc.Bacc("TRN2", target_bir_lowering=False, debug=False)

    x = nc.dram_tensor("x", [S, D], F32, kind="ExternalInput")
    wq = nc.dram_tensor("wq", [D, F], F32, kind="ExternalInput")
    wk = nc.dram_tensor("wk", [D, F], F32, kind="ExternalInput")
    wv = nc.dram_tensor("wv", [D, F], F32, kind="ExternalInput")
    wout = nc.dram_tensor("wout", [F, DO], F32, kind="ExternalInput")
    y = nc.dram_tensor("y", [S, DO], F32, kind="ExternalOutput")

    with tile.TileContext(nc) as tc, ExitStack() as top:  # noqa: PLR1702
        const_pool = top.enter_context(tc.tile_pool(name="const", bufs=1))
        ident = const_pool.tile([P, P], F32, tag="ident")
        masks.make_identity(nc, ident[:])
        ident_b = const_pool.tile([P, P], BF16, tag="identb")
        nc.vector.tensor_copy(ident_b[:], ident[:])
        ones_f32 = const_pool.tile([P, HD], F32, tag="ones_f32")
        nc.gpsimd.memset(ones_f32[:], 1.0)

        v_pool = top.enter_context(tc.tile_pool(name="vsb", bufs=1))
        v_sb = [
            v_pool.tile([P, HN * HV], dt_e, tag=f"v{st}", name=f"v_sb{st}")
            for st in range(n_st)
        ]
        for st in range(n_st):
            nc.vector.tensor_copy(
                v_sb[st][:].rearrange("p (h v) -> p h v", v=HV)[:, :, HD:].rearrange(
                    "p h one -> p (h one)"
                ),
                ones_f32[:, :HN],
            )

        outT_pool = top.enter_context(tc.tile_pool(name="outT", bufs=1))
        outP = [
            outT_pool.tile([P, S], dt_o, tag=f"o{ft}", name=f"outP{ft}")
            for ft in range(n_ft)
        ]

        wqk_pool = top.enter_context(tc.tile_pool(name="wqk", bufs=1))
        # one slab per (pair, q|k): [128, n_dt*128] with free = (db, col) —
        # a single DMA instead of n_dt small ones (SWDGE queue serializes)
        wq_s = [
            wqk_pool.tile([P, n_dt * P], dt_x, tag=f"wqs{ft}", name=f"wqs{ft}")
            for ft in range(n_ft)
        ]
        wk_s = [
            wqk_pool.tile([P, n_dt * P], dt_x, tag=f"wks{ft}", name=f"wks{ft}")
            for ft in range(n_ft)
        ]
        wq_t = [[wq_s[ft][:, db * P : (db + 1) * P] for ft in range(n_ft)] for db in range(n_dt)]
        wk_t = [[wk_s[ft][:, db * P : (db + 1) * P] for ft in range(n_ft)] for db in range(n_dt)]

        def load_w_pair(ft):
            nc.gpsimd.dma_start(
                wq_s[ft][:].rearrange("p (db c) -> p db c", c=P),
                wq[:, ft * P : (ft + 1) * P].rearrange("(db p) c -> p db c", p=P),
            )
            nc.gpsimd.dma_start(
                wk_s[ft][:].rearrange("p (db c) -> p db c", c=P),
                wk[:, ft * P : (ft + 1) * P].rearrange("(db p) c -> p db c", p=P),
            )

        # all four pairs stay resident (q-half-outer pass order reuses them)
        pair_pool = top.enter_context(tc.tile_pool(name="pair", bufs=n_ft))
        pair_tiles = {}

        def get_pair(ft):
            if ft not in pair_tiles:
                pair_tiles[ft] = (
                    pair_pool.tile([P, S], dt_qk, tag="qp", name=f"qTp{ft}"),
                    pair_pool.tile([P, S], dt_qk, tag="kp", name=f"kTp{ft}"),
                )
            return pair_tiles[ft]

        wv_pool = top.enter_context(tc.tile_pool(name="wvp", bufs=1))
        wv_t = [
            wv_pool.tile([P, F], dt_x, tag=f"wv{db}", name=f"wv{db}")
            for db in range(n_dt)
        ]
        # x^T kept resident in SBUF for the whole run (no DRAM roundtrip)
        xT_pool = top.enter_context(tc.tile_pool(name="xTsb", bufs=1))
        xT_sb = [
            xT_pool.tile([P, S], dt_x, tag=f"xT{db}", name=f"xTsb{db}")
            for db in range(n_dt)
        ]
        # upfront weight DMAs: only what the upfront phase + first passes
        # need (pair0, wv, pair1); pairs 2-3 + wout are issued later so they
        # don't compete with the x loads for HBM bandwidth
        load_w_pair(0)
        for db in range(n_dt):
            nc.gpsimd.dma_start(wv_t[db][:], wv[db * P : (db + 1) * P, :])
        load_w_pair(1)

        ps_sc = top.enter_context(
            tc.tile_pool(name="ps_sc", bufs=3, space=bass.MemorySpace.PSUM)
        )
        e_pool = top.enter_context(tc.tile_pool(name="epool", bufs=4))
        stg_pool = top.enter_context(tc.tile_pool(name="stgpool", bufs=3))
        rc_pool = top.enter_context(tc.tile_pool(name="rcpool", bufs=2))
        bcs_pool = top.enter_context(tc.tile_pool(name="bcspool", bufs=2))
        # x row staging + a dedicated transpose PSUM ring, only live during
        # the upfront phase; created last so they can be released (stack
        # order) before ps_pv/wo/ys. The transpose ring borrows the 2 PSUM
        # banks that ps_pv takes over for the pass phase.
        xup_stack = ExitStack()
        xst_pool = xup_stack.enter_context(tc.tile_pool(name="xst", bufs=2 * n_sti))
        ps_tp = xup_stack.enter_context(
            tc.tile_pool(name="ps_tp", bufs=2, space=bass.MemorySpace.PSUM)
        )

        # ---------------- building blocks ----------------
        def upfront_chunk(ch, qTp0, kTp0):
            xrows = []
            for sti in range(n_sti):
                st = ch * n_sti + sti
                xrow = xst_pool.tile([P, D], F32, tag="xrow", name=f"xrow{st}")
                nc.sync.dma_start(xrow[:], x[st * P : (st + 1) * P, :])
                xb = xst_pool.tile(
                    [P, D], dt_x, tag="xbf", bufs=2 * n_sti, name=f"xb{st}"
                )
                nc.vector.tensor_copy(xb[:], xrow[:])
                xrows.append(xb)
            xT = [xT_sb[db][:, ch * QCH : (ch + 1) * QCH] for db in range(n_dt)]
            for db in range(n_dt):
                tp = ps_tp.tile([P, QCH], dt_x, tag="tp", name=f"tr{ch}_{db}")
                for sti in range(n_sti):
                    nc.tensor.transpose(
                        tp[:, sti * P : (sti + 1) * P],
                        xrows[sti][:, db * P : (db + 1) * P],
                        ident_b[:],
                    )
                # PSUM->SBUF drain on the ACT engine (idle during upfront;
                # GPSIMD cannot access PSUM)
                nc.scalar.copy(xT[db], tp[:])
            for w_t, dstp in ((wq_t, qTp0), (wk_t, kTp0)):
                pp = ps_sc.tile([P, QCH], F32, tag="sc", name=f"pj0_{ch}")
                for db in range(n_dt):
                    nc.tensor.matmul(
                        pp[:],
                        w_t[db][0],
                        xT[db],
                        start=(db == 0),
                        stop=(db == n_dt - 1),
                    )
                nc.vector.tensor_copy(dstp[:, ch * QCH : (ch + 1) * QCH], pp[:])
            for sti in range(n_sti):
                st = ch * n_sti + sti
                pv_ps = ps_sc.tile([P, F], F32, tag="sc", name=f"pvp{st}")
                for db in range(n_dt):
                    nc.tensor.matmul(
                        pv_ps[:],
                        xT_sb[db][:, st * P : (st + 1) * P],
                        wv_t[db][:],
                        start=(db == 0),
                        stop=(db == n_dt - 1),
                    )
                nc.vector.tensor_copy(
                    v_sb[st][:].rearrange("p (h v) -> p h v", v=HV)[:, :, :HD],
                    pv_ps[:].rearrange("p (h d) -> p h d", d=HD),
                )

        def proj_items(ftn):
            """Matmul closures projecting pair ftn's qT/kT from resident xT."""
            qTp, kTp = get_pair(ftn)

            def mm_item(ch, w_t, dstp, which):
                def run():
                    pp = ps_sc.tile([P, QCH], F32, tag="sc", name=f"pj{which}{ftn}_{ch}")
                    for db in range(n_dt):
                        nc.tensor.matmul(
                            pp[:],
                            w_t[db][ftn],
                            xT_sb[db][:, ch * QCH : (ch + 1) * QCH],
                            start=(db == 0),
                            stop=(db == n_dt - 1),
                        )
                    nc.vector.tensor_copy(dstp[:, ch * QCH : (ch + 1) * QCH], pp[:])

                return run

            items = []
            for ch in range(n_ch):
                items.append(mm_item(ch, wk_t, kTp, "k"))
                items.append(mm_item(ch, wq_t, qTp, "q"))
            return items

        class AttnQH:
            """Emitter for one (pair, q-half, head-parity) attention pass,
            driven one k-tile at a time by the global stream. PV trails QK by
            LAG k-tiles so the ACT exp chain never stalls the PE."""

            LAG = 2

            def __init__(self, ft, qh, parity):
                self.ft, self.qh, self.parity = ft, qh, parity
                self.qTp, self.kTp = get_pair(ft)
                self.h = 2 * ft + parity
                self.q_base = qh * QH
                self.pv = ps_pv.tile(
                    [HV, QH], F32, tag="pv", name=f"pv{ft}_{qh}_{parity}"
                )
                self.prevs = deque()

            def emit_qk(self, kt):
                sub = self.parity * HD
                sc = ps_sc.tile(
                    [P, QH], F32, tag="sc",
                    name=f"sc{self.ft}{self.parity}{self.qh}{kt}",
                )
                for j in range(n_j):
                    q0 = self.q_base + j * QCH
                    nc.tensor.matmul(
                        sc[:, j * QCH : (j + 1) * QCH],
                        self.kTp[sub : sub + HD, kt * P : (kt + 1) * P],
                        self.qTp[sub : sub + HD, q0 : q0 + QCH],
                        start=True,
                        stop=True,
                    )
                et = e_pool.tile(
                    [P, QH], dt_e, tag="et",
                    name=f"e{self.ft}{self.parity}{self.qh}{kt}",
                )
                nc.scalar.activation(et[:], sc[:], EXP, scale=scale)
                self.prevs.append((kt, et))

            def emit_pv_one(self):
                kt, et = self.prevs.popleft()
                vt = v_sb[kt][:].rearrange("p (hh v) -> p hh v", v=HV)[:, self.h, :]
                for j in range(n_j):
                    nc.tensor.matmul(
                        self.pv[:, j * QCH : (j + 1) * QCH],
                        vt,
                        et[:, j * QCH : (j + 1) * QCH],
                        start=(kt == 0),
                        stop=(kt == n_kt - 1),
                    )
                return not self.prevs

            def finish_stage1(self):
                """Copy pv to SBUF staging (frees the PSUM accumulator)."""
                ft, qh, parity = self.ft, self.qh, self.parity
                self.stg = stg_pool.tile(
                    [HV, QH], F32, tag="stg", name=f"st{ft}{parity}{qh}"
                )
                nc.vector.tensor_copy(self.stg[:], self.pv[:])

            def normalize_items(self):
                """Per-chunk normalize closures (reciprocal + broadcast +
                multiply); DVE/GPSIMD work, no PE instructions."""
                ft, qh, parity, q_base = self.ft, self.qh, self.parity, self.q_base
                stg = self.stg

                def norm_item(qc):
                    def run():
                        rc = rc_pool.tile(
                            [1, QCH], F32, tag="rc", name=f"rc{ft}{parity}{qh}{qc}"
                        )
                        nc.vector.reciprocal(
                            rc[:], stg[HD : HD + 1, qc * QCH : (qc + 1) * QCH]
                        )
                        bcs = bcs_pool.tile(
                            [HD, QCH], F32, tag="bcs", name=f"bc{ft}{parity}{qh}{qc}"
                        )
                        nc.gpsimd.partition_broadcast(bcs[:], rc[:])
                        with nc.allow_low_precision(reason="attn out cast"):
                            nc.vector.tensor_mul(
                                outP[ft][
                                    parity * HD : (parity + 1) * HD,
                                    q_base + qc * QCH : q_base + (qc + 1) * QCH,
                                ],
                                stg[:HD, qc * QCH : (qc + 1) * QCH],
                                bcs[:],
                            )

                    return run

                return [norm_item(qc) for qc in range(n_j)]

        def y_items(qt_range, wo_t, ys_pool):
            def y_item(qt):
                def run():
                    for no in range(n_no):
                        yp = ps_sc.tile([P, QCH], F32, tag="sc", name=f"yp{qt}_{no}")
                        for ft in range(n_ft):
                            nc.tensor.matmul(
                                yp[:],
                                outP[ft][:, qt * P : (qt + 1) * P],
                                wo_t[ft][:, no * QCH : (no + 1) * QCH],
                                start=(ft == 0),
                                stop=(ft == n_ft - 1),
                            )
                        ys = ys_pool.tile([P, QCH], F32, tag="ys", name=f"ys{qt}_{no}")
                        nc.vector.tensor_copy(ys[:], yp[:])
                        nc.sync.dma_start(
                            y[qt * P : (qt + 1) * P, no * QCH : (no + 1) * QCH], ys[:]
                        )

                return run

            return [y_item(qt) for qt in qt_range]

        # ---------------- emission ----------------
        qTp0, kTp0 = get_pair(0)
        for ch in range(n_ch):
            upfront_chunk(ch, qTp0, kTp0)
        xup_stack.close()
        ps_pv = top.enter_context(
            tc.tile_pool(name="ps_pv", bufs=1, space=bass.MemorySpace.PSUM)
        )

        # late weight DMAs: pairs 2..n_ft-1 and wout
        for ftn in range(2, n_ft):
            load_w_pair(ftn)
        wo_pool = top.enter_context(tc.tile_pool(name="wo", bufs=1))
        ys_pool = top.enter_context(tc.tile_pool(name="ys", bufs=3))
        wo_t = [
            wo_pool.tile([P, DO], dt_o, tag=f"wo{ft2}", name=f"wo{ft2}")
            for ft2 in range(n_ft)
        ]
        for ft2 in range(n_ft):
            nc.gpsimd.dma_start(wo_t[ft2][:], wout[ft2 * P : (ft2 + 1) * P, :])

        # global sprinkle FIFO of (must_emit_before_pass_idx, closure)
        work = deque()
        for ftn in range(1, n_ft):
            for it in proj_items(ftn):
                work.append((2 * ftn, it))

        rows_per_qh = n_st // n_qh
        pass_specs = [
            (qh, ft, parity)
            for qh in range(n_qh)
            for ft in range(n_ft)
            for parity in (0, 1)
        ]
        n_passes = len(pass_specs)
        NEVER = n_passes + 1

        prev = None
        pending_norms = []
        enqueue_next = []  # items appended to `work` at the next pass boundary
        for pidx, (qh, ft, parity) in enumerate(pass_specs):
            for it in enqueue_next:
                work.append((NEVER, it))
            enqueue_next = []
            # force-emit overdue prerequisites (pair projections)
            while work and work[0][0] <= pidx:
                work.popleft()[1]()
            a = AttnQH(ft, qh, parity)
            stride = 3 if qh == 0 else 14
            for kt in range(n_kt):
                if pending_norms and kt >= 2:
                    pending_norms.pop(0)()
                if work and kt % stride == stride - 1:
                    work.popleft()[1]()
                a.emit_qk(kt)
                if prev is not None:
                    if prev.emit_pv_one():
                        prev.finish_stage1()
                        pending_norms += prev.normalize_items()
                        if (prev.qh, prev.ft, prev.parity) == (0, n_ft - 1, 1):
                            # first q-half fully normalized soon: release its
                            # out-projection rows into the stream
                            enqueue_next += y_items(
                                range(rows_per_qh), wo_t, ys_pool
                            )
                        prev = None
                elif len(a.prevs) > AttnQH.LAG:
                    a.emit_pv_one()
            prev = a

        # tail: flush the last pass, run leftovers, last normalizes, last rows
        while prev.prevs:
            if work:
                work.popleft()[1]()
            prev.emit_pv_one()
        prev.finish_stage1()
        tail_norms = pending_norms + prev.normalize_items()
        leftovers = [it for _, it in work]
        for i, it in enumerate(leftovers[:2]):
            if tail_norms:
                tail_norms.pop(0)()
            it()
        for it in tail_norms:
            it()
        for it in leftovers[2:]:
            it()
        for it in y_items(range(rows_per_qh, n_st), wo_t, ys_pool):
            it()

    nc.compile()
    return nc


# problem sizes (hardcoded per contract)
B, S, D, H = 4, 2048, 1024, 16
DO = D
HN = H // 2  # heads per core
SCALE = (D // H) ** -0.5
N_CORES = 8

_NC_CACHE = None


def _get_nc():
    global _NC_CACHE
    if _NC_CACHE is None:
        _NC_CACHE = build_attention(S, D, HN, DO, SCALE)
    return _NC_CACHE


def make_in_maps(x, w_qkv, w_out):
    """Shard full inputs into the 8 per-core input maps."""
    in_maps = []
    for c in range(N_CORES):
        b = c // 2
        cs = (c % 2) * HN * HD
        ce = cs + HN * HD
        in_maps.append(
            {
                "x": np.ascontiguousarray(x[b]),
                "wq": np.ascontiguousarray(w_qkv[:, cs:ce]),
                "wk": np.ascontiguousarray(w_qkv[:, D + cs : D + ce]),
                "wv": np.ascontiguousarray(w_qkv[:, 2 * D + cs : 2 * D + ce]),
                "wout": np.ascontiguousarray(w_out[cs:ce, :]),
            }
        )
    return in_maps


def combine_outputs(results, b_out):
    """Sum the two per-batch partials and add the bias."""
    y = np.empty((B, S, DO), dtype=np.float32)
    for b in range(B):
        y[b] = results[2 * b]["y"] + results[2 * b + 1]["y"] + b_out[None, :]
    return y


def kernel(x, w_qkv, w_out, b_out):
    x = np.asarray(x, dtype=np.float32)
    w_qkv = np.asarray(w_qkv, dtype=np.float32)
    w_out = np.asarray(w_out, dtype=np.float32)
    b_out = np.asarray(b_out, dtype=np.float32)
    nc = _get_nc()
    in_maps = make_in_maps(x, w_qkv, w_out)
    res = bass_utils.run_bass_kernel_spmd(nc, in_maps, core_ids=list(range(N_CORES)))
    return combine_outputs(res.results, b_out)
